# revision 17
# baseline (speedup 1.0000x reference)
"""GaborConv2d Trainium2 kernel.

Strategy
--------
Host: generate the tiny [64,3,7,7] Gabor weights from (freq, theta, sigma,
psi), pad the input, and build a row-shared im2col stack: for each group of
G=4 output rows (a "gb" block), 60 planes p = ri*6 + c*2 + u (ri in 0..9,
u in {0,1}) hold pad[c, gb*4+ri, x+u].  Output row j (0..3) of the block
contracts the plane window [6j .. 6j+41] — consecutive rows share input
planes, so HBM input traffic is 60/4 = 15 planes/row instead of 42
(15.9 MB/core instead of 44.5).  Windows are read as K = 42+6j from the
strip-aligned base 64h (the BIR verifier rejects unaligned partition
bases); the 6j leading weight rows are zero, which is free since matmul
cost depends only on N.

Device (per core, 2 images batch-sharded): tiles of 8 gb blocks (32 output
rows).  Two partition halves (64h) hold alternating gb blocks; matmuls are
4 accumulating supertaps (kj = 2t+u) with K<=60, M=64, N=512 on rotating
64x64 quadrants (row half 64h x col half 64dj), keeping 4 streams in
flight.  A row pair (r, r+16) accumulates into one PSUM bank (partitions
0-63 / 64-127), evicted by a single [128,512] f32->bf16 copy (VectorE and
ScalarE alternating) into a 32-row staging tile, stored as two 1 MB bf16
transfers with 16 KB contiguous runs (the measured store sweet spot).
Input loads stay on the scalar HWDGE queue, stores on sync — sharing one
queue serializes them.  An ~5 us dense-matmul warm-up at kernel start
brings the PE out of the 1.2 GHz HAM-throttled state before real work.
Output is bf16, upcast on host (rel-err ~4e-3 vs the 2e-2 budget).

Measured on the 8-core axon trn2 pod: 277 us HW exec (core 0), vs 651 us
for the previous 42-plane f32-output version.
"""

import math

import ml_dtypes
import numpy as np

import concourse.bass as bass
import concourse.mybir as mybir
import concourse.tile as tile
from concourse import bacc
from concourse.bass_utils import run_bass_kernel_spmd

F32 = mybir.dt.float32
BF16 = mybir.dt.bfloat16
BF16NP = ml_dtypes.bfloat16

N_CORES = 8
B, C, H, W = 16, 3, 512, 512
O, K, PAD = 64, 7, 3
IPC = B // N_CORES          # images per core
G = 4                       # output rows per gb block
NPL = 60                    # planes per gb block: (6+G)*6
NWIN = 42                   # planes per output-row window
XW = 518                    # stored plane width (512 + 6 taps)
NTAP = 4                    # supertaps, kj = 2t+u
NGB = H // G                # 128 gb blocks per image
GBT = 8                     # gb blocks per SBUF tile (32 rows)
NTILE = NGB // GBT          # 16 tiles per image
DELTA = 0.001


def _gabor_weights(freq, theta, sigma, psi):
    x0 = math.ceil(K / 2)
    lin = np.linspace(-x0 + 1, x0, K, dtype=np.float32)
    y = np.broadcast_to(lin[:, None], (K, K))
    x = np.broadcast_to(lin[None, :], (K, K))
    th = theta[:, :, None, None].astype(np.float32)
    fr = freq[:, :, None, None].astype(np.float32)
    sg = sigma[:, :, None, None].astype(np.float32)
    ps = psi[:, :, None, None].astype(np.float32)
    rotx = x * np.cos(th) + y * np.sin(th)
    roty = -x * np.sin(th) + y * np.cos(th)
    g = np.exp(-0.5 * ((rotx**2 + roty**2) / (sg + DELTA) ** 2))
    g = g * np.cos(fr * rotx + ps)
    g = g / (2 * np.pi * sg**2)
    return g.astype(np.float32)  # [O, C, K, K]


def _build_nc():
    nc = bacc.Bacc(None, target_bir_lowering=False)
    # xstack2[img, tl, h, p, s, x]: plane p of gb block tl*8 + 2s + h
    xs = nc.dram_tensor(
        "xstack", [IPC, NTILE, 2, NPL, 4, XW], BF16, kind="ExternalInput"
    )
    wb = nc.dram_tensor("wbig", [128, G * NTAP * O], BF16, kind="ExternalInput")
    # fully-contiguous store layout, un-permuted on the host:
    # ydev[img, tl, hh*64+o, k, x] = out[img, o, tl*32 + hh*16 + k, x]
    y = nc.dram_tensor("y", [IPC, NTILE, 128, 16 * W], BF16, kind="ExternalOutput")
    junk = nc.dram_tensor("junk", [128, 4], F32, kind="ExternalOutput")

    with tile.TileContext(nc) as tc:
        with (
            tc.tile_pool(name="wpool", bufs=1) as wpool,
            tc.tile_pool(name="ipool", bufs=8) as ipool,
            tc.tile_pool(name="spool", bufs=3) as spool,
            tc.tile_pool(name="ppool", bufs=8, space="PSUM") as ppool,
        ):
            wt = wpool.tile([128, G * NTAP * O], BF16)
            nc.sync.dma_start(out=wt, in_=wb[:])

            # HAM warm-up: ~5us of dense full-array matmuls on garbage data
            # so the PE clock is at 2.4GHz before real work starts.  The
            # junk output keeps the chain from being dead-code eliminated.
            dps = ppool.tile([128, W], F32, tag="ps")
            for wi in range(8):
                nc.tensor.matmul(
                    dps[:, :],
                    wt[:, 0:128],
                    wt[:, 0:W],
                    start=(wi == 0),
                    stop=(wi == 7),
                )
            jt = wpool.tile([128, 4], F32)
            nc.vector.tensor_copy(jt, dps[:, 0:4])
            nc.sync.dma_start(out=junk[:], in_=jt)

            for img in range(IPC):
                for tl in range(NTILE):
                    it = ipool.tile([128, 4 * XW], BF16, tag="img")
                    for h in range(2):
                        # partitions 64h..64h+59 <- gb blocks tl*8 + 2s + h
                        nc.scalar.dma_start(
                            out=it[64 * h : 64 * h + NPL, :],
                            in_=bass.AP(
                                xs,
                                ((img * NTILE + tl) * 2 + h) * NPL * 4 * XW,
                                [[4 * XW, NPL], [1, 4 * XW]],
                            ),
                        )
                    stg = spool.tile([128, 16 * W], BF16, tag="stg")
                    pidx = 0
                    for s2 in range(2):
                        for j in range(G):
                            for h in range(2):
                                # pair rows r and r+16: dj=0 -> gb slot s2,
                                # dj=1 -> slot s2+2 (same window j, same h)
                                ps = ppool.tile([128, W], F32, tag="ps")
                                # window base must be strip-aligned: start at
                                # 64h, widen K by 6j zero weight rows
                                kk = NWIN + 6 * j
                                for t in range(NTAP):
                                    for dj in range(2):
                                        s = s2 + 2 * dj
                                        nc.tensor.matmul(
                                            ps[64 * dj : 64 * dj + O, :],
                                            wt[
                                                64 * h : 64 * h + kk,
                                                (G * j + t) * O : (G * j + t + 1) * O,
                                            ],
                                            it[
                                                64 * h : 64 * h + kk,
                                                s * XW + 2 * t : s * XW + 2 * t + W,
                                            ],
                                            start=(t == 0),
                                            stop=(t == NTAP - 1),
                                            tile_position=(64 * h, 64 * dj),
                                        )
                                slot = 8 * s2 + 4 * h + j
                                sl = stg[:, slot * W : (slot + 1) * W]
                                # vector takes 9/16: scalar also issues the
                                # input-load DMAs, so balance engine time
                                if pidx % 16 < 9:
                                    nc.vector.tensor_copy(sl, ps[:, :])
                                else:
                                    nc.scalar.copy(sl, ps[:, :])
                                pidx += 1
                    # one 2MB store, sequential DRAM addresses
                    nc.sync.dma_start(
                        out=bass.AP(
                            y,
                            (img * NTILE + tl) * 128 * 16 * W,
                            [[16 * W, 128], [1, 16 * W]],
                        ),
                        in_=stg[:, :],
                    )
    nc.finalize()
    return nc


def _prepare_inputs(input_tensor, freq, theta, sigma, psi):
    g = _gabor_weights(freq, theta, sigma, psi)  # [O, C, K, K] f32
    # wbig[64h + 6j + (6kr + 2c + u), (4j + t)*64 + o] = g[o, c, kr, 2t+u]
    wmat = np.zeros((128, G * NTAP * O), np.float32)
    for h in range(2):
        for j in range(G):
            for t in range(NTAP):
                for kr in range(K):
                    for c in range(C):
                        for u in range(2):
                            kj = 2 * t + u
                            if kj >= K:
                                continue
                            p = 64 * h + 6 * j + 6 * kr + 2 * c + u
                            wmat[p, (G * j + t) * O : (G * j + t + 1) * O] = g[
                                :, c, kr, kj
                            ]
    wbig = wmat.astype(BF16NP)

    xb = input_tensor.astype(BF16NP)
    pad = np.zeros((B, C, H + 2 * PAD, W + 2 * PAD + 2), BF16NP)
    pad[:, :, PAD : PAD + H, PAD : PAD + W] = xb
    # xstack[img, gb, ri*6 + c*2 + u, x] = pad[img, c, gb*4 + ri, x+u]
    xstack = np.empty((B, NGB, NPL, XW), BF16NP)
    for ri in range(6 + G):
        for c in range(C):
            for u in range(2):
                p = ri * 6 + c * 2 + u
                xstack[:, :, p, :] = pad[:, c, ri : ri + 4 * NGB : G, u : u + XW][
                    :, :NGB, :
                ]
    # xstack2[img, tl, h, p, s, x] = xstack[img, tl*8 + 2s + h, p, x]
    xstack2 = np.ascontiguousarray(
        xstack.reshape(B, NTILE, 4, 2, NPL, XW).transpose(0, 1, 3, 4, 2, 5)
    )
    in_maps = [
        {"xstack": xstack2[core * IPC : (core + 1) * IPC], "wbig": wbig}
        for core in range(N_CORES)
    ]
    return in_maps


_NC_CACHE = None


def kernel(input_tensor, freq, theta, sigma, psi):
    global _NC_CACHE
    input_tensor = np.asarray(input_tensor, dtype=np.float32)
    in_maps = _prepare_inputs(
        input_tensor,
        np.asarray(freq), np.asarray(theta), np.asarray(sigma), np.asarray(psi),
    )
    if _NC_CACHE is None:
        _NC_CACHE = _build_nc()
    res = run_bass_kernel_spmd(_NC_CACHE, in_maps, core_ids=list(range(N_CORES)))
    out = np.concatenate([r["y"] for r in res.results], axis=0)
    # ydev[img, tl, hh*64+o, k, x] -> y[img, o, tl*32 + hh*16 + k, x]
    out = (
        out.reshape(B, NTILE, 2, O, 16, W)
        .transpose(0, 3, 1, 2, 4, 5)
        .reshape(B, O, H, W)
    )
    return out.astype(np.float32)
